# revision 54
# baseline (speedup 1.0000x reference)
"""Trainium2 Bass kernel v3: multi-head self-attention with RoPE
(B=2, S=2048, d_model=1024, 16 heads, causal) over 8 NeuronCores.

Sharding: core c handles batch c//4 and head-group c%4 (4 heads); host
sums the 4 partial output projections per batch element.

v3 vs v2:
- all matmul operands bf16 (same PE rate as f32r, no >=256-column rule,
  half the DMA/SBUF traffic)
- AV matmul in fp8e4 DoubleRow over paired k-tiles: P = exp(s/8 - 2)
  (constant softmax shift keeps exp inside e4m3 range; the shift cancels
  in the normalization), V in fp8e4
- single Q/K projection; RoPE = ps*cos + swap32(ps*sin_sw) with the
  e/o 32-partition swap done by SBUF->SBUF DMA (PE saves the duplicate
  e/o-swapped projections v2 needed)
- attend iterates (k-tile, 256-wide q half); next block's projections
  and the previous block's output projection are interleaved into the
  attend loop as PE filler work so the PE never waits on the Act
  engine's exp stream
- PSUM: shared "big" tag 2x(4KB/partition) + av 4x(2KB/partition)
  = exactly 8 banks
"""

from contextlib import ExitStack

import numpy as np

import concourse.bass as bass
import concourse.bacc as bacc
import concourse.tile as tile
import concourse.mybir as mybir

F32 = mybir.dt.float32
BF = mybir.dt.bfloat16
F8 = mybir.dt.float8e4
EXP = mybir.ActivationFunctionType.Exp
DR = mybir.MatmulPerfMode.DoubleRow

D = 1024          # d_model
HPC = 4           # heads per core
DK = 64
DL = HPC * DK     # 256 local v-channels
QB = 512          # q block
HB = 256          # attend half-block (psum granularity)
KB = 128          # k tile
NDC = D // 128    # 8 contraction chunks
CSH = 0.0         # softmax shift (needed only for fp8 P; bf16 needs none)
FP8AV = False     # fp8 DoubleRow AV (fast) vs bf16 single-k-tile AV (safe)
SEQFILL = False   # interleave fillers into attend (the scheduling win)
PDT = F8 if FP8AV else BF


def build_nc(S=2048, mm_dtype="bf16", repeat=1):
    assert mm_dtype == "bf16"
    nq = S // QB
    nkt = S // KB
    nc = bacc.Bacc("TRN2", target_bir_lowering=False, debug=False,
                   enable_asserts=True)
    xT = nc.dram_tensor("xT", [D, S], BF, kind="ExternalInput").ap()
    wqT = nc.dram_tensor("wqT", [128, NDC * 256], BF,
                         kind="ExternalInput").ap()
    wkT = nc.dram_tensor("wkT", [128, NDC * 256], BF,
                         kind="ExternalInput").ap()
    wvT = nc.dram_tensor("wvT", [128, NDC * DL], BF,
                         kind="ExternalInput").ap()
    woT = nc.dram_tensor("woT", [128, 2 * D], BF, kind="ExternalInput").ap()
    cosb = nc.dram_tensor("cosb", [128, S], BF, kind="ExternalInput").ap()
    snswb = nc.dram_tensor("snswb", [128, S], BF, kind="ExternalInput").ap()
    tri4 = nc.dram_tensor("tri4", [128, HPC * KB], PDT,
                          kind="ExternalInput").ap()
    out = nc.dram_tensor("out", [S, D], BF, kind="ExternalOutput").ap()

    with tile.TileContext(nc) as tc, ExitStack() as ctx, \
            nc.allow_low_precision(reason="bf16 matmuls, fp8 AV, f32 psum"):
        const = ctx.enter_context(tc.tile_pool(name="const", bufs=1))
        xtp = ctx.enter_context(tc.tile_pool(name="xtp", bufs=2))
        qkp = ctx.enter_context(tc.tile_pool(name="qkp", bufs=2))
        qp = ctx.enter_context(tc.tile_pool(name="qp", bufs=2))
        rp = ctx.enter_context(tc.tile_pool(name="rp", bufs=2))
        vap = ctx.enter_context(tc.tile_pool(name="vap", bufs=nkt))
        ptp = ctx.enter_context(tc.tile_pool(name="ptp", bufs=16))
        otp = ctx.enter_context(tc.tile_pool(name="otp", bufs=4))
        smp = ctx.enter_context(tc.tile_pool(name="smp", bufs=4))
        osp = ctx.enter_context(tc.tile_pool(name="osp", bufs=1))
        pp = ctx.enter_context(tc.tile_pool(name="pp", bufs=1, space="PSUM"))

        # ---- constants / weights ----
        wq_sb = const.tile([128, NDC, 256], BF, name="wq_sb", tag="wq_sb")
        wq_r = wqT.rearrange("p (c m) -> p c m", c=NDC)
        nc.sync.dma_start(out=wq_sb[:, 0:1, :], in_=wq_r[:, 0:1, :])
        nc.sync.dma_start(out=wq_sb[:, 1:NDC, :], in_=wq_r[:, 1:NDC, :])
        wk_sb = const.tile([128, NDC, 256], BF, name="wk_sb", tag="wk_sb")
        nc.scalar.dma_start(out=wk_sb,
                            in_=wkT.rearrange("p (c m) -> p c m", c=NDC))
        cs_sb = const.tile([128, S], BF, name="cs_sb", tag="cs_sb")
        nc.scalar.dma_start(out=cs_sb, in_=cosb)
        sn_sb = const.tile([128, S], BF, name="sn_sb", tag="sn_sb")
        nc.scalar.dma_start(out=sn_sb, in_=snswb)
        wv_sb = const.tile([128, NDC, DL], BF, name="wv_sb", tag="wv_sb")
        tri_sb = const.tile([128, HPC, KB], PDT, name="tri_sb", tag="tri_sb")
        wo_sb = const.tile([128, 2, D], BF, name="wo_sb", tag="wo_sb")

        def load_late_consts():   # on SP, after x(0)
            nc.sync.dma_start(
                out=wv_sb, in_=wvT.rearrange("p (c m) -> p c m", c=NDC))
            nc.sync.dma_start(
                out=tri_sb, in_=tri4.rearrange("p (h k) -> p h k", h=HPC))
            nc.sync.dma_start(
                out=wo_sb, in_=woT.rearrange("p (c m) -> p c m", c=2))
        bt = const.tile([128, 1], F32, name="bt", tag="bt")
        nc.vector.memset(bt, -CSH)
        ones64 = const.tile([1, 64], BF, name="ones64", tag="ones64")
        nc.vector.memset(ones64, 1.0)
        onescol = const.tile([128, 2, 1], F8, name="onescol", tag="onescol")
        nc.vector.memset(onescol, 1.0)

        def bview(ap2d, sl, n):
            """[128, S] slice -> broadcast [128, n, w] for DVE ops."""
            return ap2d[:, sl].rearrange("p (o q) -> p o q", o=1) \
                .to_broadcast((128, n, sl.stop - sl.start))

        def load_x(st):
            # one 1MB DMA per block: all 8 contraction chunks at once
            # (partition-split rearrange is legal on the DRAM side)
            ssl = slice(st * QB, (st + 1) * QB)
            xt = xtp.tile([128, NDC, QB], BF, name="xt", tag="xt")
            nc.sync.dma_start(
                out=xt, in_=xT[:, ssl].rearrange("(c p) q -> p c q", p=128))
            return xt

        # ---------------- projections + rope ----------------

        def proj_qk_units(xt, w_sb, st, q2dst, q2bdst):
            """Returns (filler_units, finalize) for one Q/K projection of
            block st. Each unit emits 8 matmuls (~0.85us PE); the rope
            chain (DVE muls, DMA swap, DVE add) is emitted by the last
            unit of each HB sub-chunk."""
            ssl = slice(st * QB, (st + 1) * QB)
            u = rp.tile([128, 2, QB], BF, name="u", tag="u")
            t1 = rp.tile([128, 2, QB], BF, name="t1", tag="t1")
            us = rp.tile([128, 2, QB], BF, name="us", tag="us")
            units = []

            def emit_chunk(sub, t):
                def f():
                    # self-contained ~0.85us unit: its own 1KB psum tile
                    # (own zero region), group opened+closed inside, rope
                    # muls drain it immediately. Fillers must never hold a
                    # "big" slot across other units (rotation deadlock).
                    ps = pp.tile([128, HB], F32, name="ps",
                                 tag="fil", bufs=2)
                    qsl = slice(sub * HB, (sub + 1) * HB)
                    for dc in range(NDC):
                        nc.tensor.matmul(
                            ps, w_sb[:, dc, 128 * t:128 * (t + 1)],
                            xt[:, dc, qsl],
                            start=(dc == 0), stop=(dc == NDC - 1))
                    gsl = slice(ssl.start + sub * HB,
                                ssl.start + (sub + 1) * HB)
                    nc.vector.tensor_mul(u[:, t, qsl], ps, sn_sb[:, gsl])
                    nc.vector.tensor_mul(t1[:, t, qsl], ps, cs_sb[:, gsl])
                return f

            for sub in range(QB // HB):
                for t in range(2):
                    units.append((0.85, emit_chunk(sub, t)))

            def finalize():
                # e/o 32-block swap of u on the SP hwdge queue (x loads are
                # prefetched well ahead, so queueing behind them is safe;
                # keeps DMA issue off the Act and Pool sequencers)
                for base in (0, 64):
                    nc.sync.dma_start(out=us[base:base + 32],
                                      in_=u[base + 32:base + 64])
                    nc.sync.dma_start(out=us[base + 32:base + 64],
                                      in_=u[base:base + 32])
                nc.vector.tensor_add(q2dst, t1, us)
                # bf16 Ldweights with tile_position row 64 wedges the device
                # (f32r self-loads and never hit this): stage the upper 64
                # channels at base partition 0 for the score matmuls.
                nc.sync.dma_start(out=q2bdst, in_=q2dst[64:128])

            return units, (0.1, finalize)

        def proj_v_units(st, xt, vaug):
            """V projection for block st: 4 k-tiles, one unit per k-tile."""
            units = []

            def emit_kt(ss):
                def f():
                    kt = st * (QB // KB) + ss
                    psv = pp.tile([128, DL], F32, name="ps_v", tag="fil",
                                  bufs=2)
                    for dc in range(NDC):
                        nc.tensor.matmul(
                            psv, xt[:, dc, ss * KB:(ss + 1) * KB],
                            wv_sb[:, dc, :],
                            start=(dc == 0), stop=(dc == NDC - 1))
                    if kt % 2 == 0:
                        # per-head stride padded to 80B so the DoubleRow
                        # lhsT outer step (4*80=320B) is 16B-aligned; col 64
                        # holds the ones row for the softmax denominator
                        va = vap.tile([128, 2, HPC, 80], PDT, name="vaug",
                                      tag="vaug")
                        nc.vector.memset(va[:, :, :, 64:65], 1.0)
                        assert len(vaug) == kt // 2
                        vaug.append(va)
                    va = vaug[kt // 2]
                    nc.vector.tensor_copy(
                        va[:, kt % 2, :, 0:64],
                        psv.rearrange("p (h d) -> p h d", h=HPC))
                return f

            for ss in range(QB // KB):
                units.append((0.95, emit_kt(ss)))
            return units

        # ---------------- attention ----------------

        def av_pass(st, heads, vaug, pts, avtiles):
            """AV accumulation for two heads over all pairs of block st
            (fp8 DoubleRow, 65-row out with the ones-row denominator)."""
            npair = 2 * (st + 1)
            for kp in range(npair):
                first = True
                for hf in range(QB // HB):
                    ke = 2 * kp
                    goff_e = KB * (ke % 4) if (ke // 4 == st) else 0
                    aoff = min(max(goff_e - hf * HB, 0), HB)
                    if aoff >= HB:
                        continue
                    csl2 = slice(hf * HB + aoff, (hf + 1) * HB)
                    pt = pts[(kp, hf)]
                    for j, h in enumerate(heads):
                        if FP8AV:
                            nc.tensor.matmul(
                                avtiles[j][:, csl2],
                                vaug[kp][:, :, h, 0:65],
                                pt[:, :, h, aoff:],
                                start=(kp == 0 and first),
                                stop=(kp == npair - 1),
                                perf_mode=DR)
                        else:
                            for kin2 in range(2):
                                nc.tensor.matmul(
                                    avtiles[j][:, csl2],
                                    vaug[kp][:, kin2, h, 0:65],
                                    pt[:, kin2, h, aoff:],
                                    start=(kp == 0 and first and kin2 == 0),
                                    stop=(kp == npair - 1 and kin2 == 1))
                    first = False

        def attend(st, q2, q2b, k2, k2b, vaug, fillers):
            """Causal attention for q-block st; drains `fillers` (PE work
            closures) by a PE-time budget per iteration. AV runs in two
            passes of 2 heads (2 psum banks): heads 0,1 right after the
            score loop, then — once their tiles are drained — heads 2,3.
            Returns (avs SBUF tiles, rcs reciprocal tiles)."""
            avA = [pp.tile([65, QB], F32, name=f"ps_av{h}", tag="avt",
                           bufs=2) for h in range(2)]
            av_start = [True, True]
            nki = 4 * (st + 1)
            npair = nki // 2

            # iteration list: (ki, hf, loff) for alive halves
            iters = []
            for ki in range(nki):
                diag = (ki // 4 == st)
                goff = KB * (ki % 4) if diag else 0
                for hf in range(QB // HB):
                    loff = min(max(goff - hf * HB, 0), HB)
                    if loff >= HB:
                        continue
                    iters.append((ki, hf, loff, diag, goff))
            n_it = len(iters)
            done_f = [0]
            spent = [0.0]
            total_cost = sum(c for c, _ in fillers)

            def pop_fillers(i_it):
                # drain fillers by PE-time budget: aim to finish all units
                # by the last iteration, but cap the per-iteration burst so
                # the Act engine's exp stream is never starved for long.
                remaining_it = n_it - i_it
                remaining = total_cost - spent[0]
                budget = spent[0] + max(0.7, remaining / max(remaining_it, 1))
                while done_f[0] < len(fillers) and spent[0] < budget:
                    c, f = fillers[done_f[0]]
                    f()
                    spent[0] += c
                    done_f[0] += 1

            pts = {}      # (kp, hf) -> PT tile (kept for av pass B)
            for i_it, (ki, hf, loff, diag, goff) in enumerate(iters):
                kin = ki % 2
                kp = ki // 2
                ksl = slice(ki * KB, (ki + 1) * KB)
                csl = slice(hf * HB + loff, (hf + 1) * HB)
                lsl = slice(loff, HB)
                if (kp, hf) not in pts:
                    pts[(kp, hf)] = ptp.tile([128, 2, HPC, HB], PDT,
                                             name=f"pt{hf}", tag="pt")
                pt = pts[(kp, hf)]
                sct = pp.tile([128, HPC, HB], F32, name="sct", tag="sct",
                              bufs=2)
                # sct is 4KB/partition = two psum zero regions, each holding
                # a pair of heads: start/stop on the first/last head of each
                # region (a second start=True would wipe the first head).
                for h in range(HPC):
                    t, uu = h // 2, h % 2
                    ksrc = k2 if uu == 0 else k2b
                    qsrc = q2 if uu == 0 else q2b
                    nc.tensor.matmul(sct[:, h, lsl],
                                     ksrc[0:64, t, ksl],
                                     qsrc[0:64, t, csl],
                                     start=(h % 2 == 0), stop=(h % 2 == 1),
                                     tile_position=(0, 0))
                nc.scalar.activation(out=pt[:, kin, :, lsl],
                                     in_=sct[:, :, lsl],
                                     func=EXP, scale=0.125, bias=bt[:, 0:1])
                if diag and 0 <= goff - hf * HB < HB:
                    dsl = slice(loff, loff + KB)
                    nc.vector.tensor_mul(pt[:, kin, :, dsl],
                                         pt[:, kin, :, dsl], tri_sb)
                if kin == 1:
                    # close the pair on this half: zero the odd plane's
                    # dead region and run pass-A AV (heads 0,1)
                    ke = ki - 1
                    goff_e = KB * (ke % 4) if (ke // 4 == st) else 0
                    aoff = min(max(goff_e - hf * HB, 0), HB)
                    if loff > aoff:
                        nc.vector.memset(pt[:, 1, :, aoff:loff], 0.0)
                    csl2 = slice(hf * HB + aoff, (hf + 1) * HB)
                    for h in (0, 1):
                        if FP8AV:
                            nc.tensor.matmul(
                                avA[h][:, csl2],
                                vaug[kp][:, :, h, 0:65],
                                pt[:, :, h, aoff:],
                                start=av_start[h],
                                stop=(kp == npair - 1),
                                perf_mode=DR)
                        else:
                            for kin2 in range(2):
                                nc.tensor.matmul(
                                    avA[h][:, csl2],
                                    vaug[kp][:, kin2, h, 0:65],
                                    pt[:, kin2, h, aoff:],
                                    start=(av_start[h] and kin2 == 0),
                                    stop=(kp == npair - 1 and kin2 == 1))
                        av_start[h] = False
                pop_fillers(i_it)
            while done_f[0] < len(fillers):
                fillers[done_f[0]][1]()
                done_f[0] += 1

            def drain(avtiles, heads):
                """reciprocals of the ones-row denominators + psum->SBUF
                copies for a finished head pair."""
                vc = heads[0] // 2
                for j, h in enumerate(heads):
                    rc = smp.tile([1, QB], BF, name="rc", tag="rc", bufs=8)
                    nc.vector.reciprocal(rc, avtiles[j][64:65, :])
                    rcs.append(rc)
                a = otp.tile([128, QB], BF, name="avs", tag="avs", bufs=4)
                nc.vector.tensor_copy(a[0:64, :], avtiles[0][0:64, :])
                tmp = otp.tile([64, QB], BF, name="avtmp", tag="avtmp",
                               bufs=2)
                nc.vector.tensor_copy(tmp, avtiles[1][0:64, :])
                # partition shift 0-63 -> 64-127 via SBUF->SBUF DMA
                nc.sync.dma_start(out=a[64:128, :], in_=tmp)
                avs.append(a)

            rcs, avs = [], []
            drain(avA, (0, 1))
            avB = [pp.tile([65, QB], F32, name=f"ps_av{h}", tag="avt",
                           bufs=2) for h in (2, 3)]
            av_pass(st, (2, 3), vaug, pts, avB)
            drain(avB, (2, 3))
            return avs, rcs

        # ---------------- softmax denominators + output projection ----

        def finish_units(st, avs, rcs, last=False):
            """PE filler units for block st's normalization and output
            projection, interleaved into attend(st+1)."""
            bsbs = []
            units = []

            def bc_unit(vc):
                def f():
                    # broadcast 1/denom over the 128 partitions of the
                    # head pair: rows 0-63 head 2vc, rows 64-127 head 2vc+1
                    bc = pp.tile([128, QB], F32, name="ps_bc", tag="fil",
                                 bufs=2)
                    nc.tensor.matmul(bc[0:64, :], ones64, rcs[2 * vc],
                                     start=True, stop=True,
                                     tile_position=(0, 0))
                    nc.tensor.matmul(bc[64:128, :], ones64, rcs[2 * vc + 1],
                                     start=True, stop=True,
                                     tile_position=(0, 64),
                                     skip_group_check=True)
                    bsb = smp.tile([128, QB], BF, name="bsb", tag="bsb",
                                   bufs=4)
                    nc.vector.tensor_copy(bsb, bc)
                    bsbs.append(bsb)
                return f

            ot = []

            def ot_unit(vc):
                def f():
                    # SBUF-only normalize -> runs on the idle Pool engine
                    o = otp.tile([128, QB], BF, name="ot", tag="ot")
                    nc.vector.tensor_mul(o, avs[vc], bsbs[vc])
                    ot.append(o)
                return f

            def pso_unit(ss, dt_):
                def f():
                    pso = pp.tile([128, QB], F32, name="ps_o", tag="fil",
                                  bufs=2)
                    for vc in range(2):
                        nc.tensor.matmul(
                            pso,
                            ot[vc][:, ss * 128:(ss + 1) * 128],
                            wo_sb[:, vc, dt_ * QB:(dt_ + 1) * QB],
                            start=(vc == 0), stop=(vc == 1))
                    eng = (nc.vector, nc.scalar)[(2 * ss + dt_) % 2]
                    if eng is nc.scalar:
                        eng.copy(osts[:, ss, dt_, :], pso)
                    else:
                        eng.tensor_copy(osts[:, ss, dt_, :], pso)
                    if ss == QB // 128 - 1 and dt_ == 1:
                        # single 1MB store for the whole q block
                        rows = slice(st * QB, (st + 1) * QB)
                        nc.sync.dma_start(
                            out=out[rows, :].rearrange(
                                "(s p) d -> p s d", p=128),
                            in_=osts.rearrange("p s a b -> p s (a b)"))
                return f

            osts = osp.tile([128, QB // 128, 2, QB], BF, name="ost",
                            tag="ost")
            units.append((0.5, bc_unit(0)))
            units.append((0.5, bc_unit(1)))
            units.append((0.1, ot_unit(0)))
            units.append((0.1, ot_unit(1)))
            for ss in range(QB // 128):
                for dt_ in range(2):
                    units.append((0.5, pso_unit(ss, dt_)))
            return units

        # ---------------- main loop ----------------

        def prep_block(rep, st, rst, xt):
            """Build projection fillers for block (rep, st)."""
            if st == 0:
                rst["vaug"] = []
                rst["k2"] = qkp.tile([128, 2, S], BF, name="k2", tag="k2")
                rst["k2b"] = qkp.tile([64, 2, S], BF, name="k2b", tag="k2b")
            q2n = qp.tile([128, 2, QB], BF, name="q2", tag="q2")
            q2bn = qp.tile([64, 2, QB], BF, name="q2b", tag="q2b")
            ssl = slice(st * QB, (st + 1) * QB)
            uq, fq = proj_qk_units(xt, wq_sb, st, q2n, q2bn)
            uk, fk = proj_qk_units(xt, wk_sb, st, rst["k2"][:, :, ssl],
                                   rst["k2b"][:, :, ssl])
            uv = proj_v_units(st, xt, rst["vaug"])
            units = uq + [fq] + uk + [fk] + uv
            return (q2n, q2bn), units

        # blocks across all repeats; the next block's projections (and on
        # rep boundaries, the next rep's whole prologue) interleave into the
        # current attend, so steady state has no serial phases. x is
        # prefetched two blocks ahead so proj fillers never wait on DMA.
        blocks = [(rep, st) for rep in range(repeat) for st in range(nq)]
        rstates = {0: {}}
        x_tiles = {0: load_x(0)}
        (q2_cur, q2b_cur), units0 = prep_block(0, 0, rstates[0], x_tiles[0])
        load_late_consts()
        if len(blocks) > 1:
            x_tiles[1] = load_x(blocks[1][1])
        for _, un in units0:
            un()

        pending_fin = []
        for g, (rep, st) in enumerate(blocks):
            rst = rstates[rep]
            if g + 1 < len(blocks) and g + 1 not in x_tiles:
                x_tiles[g + 1] = load_x(blocks[g + 1][1])
            proj_fillers = []
            q2_next = q2b_next = None
            if g + 1 < len(blocks):
                rep1, st1 = blocks[g + 1]
                if st1 == 0:
                    rstates[rep1] = {}
                (q2_next, q2b_next), proj_fillers = prep_block(
                    rep1, st1, rstates[rep1], x_tiles.pop(g + 1))
            # previous block's finish units lead (their DVE inputs are
            # ready); projections alternate in
            fillers = []
            pf, ff = list(proj_fillers), list(pending_fin)
            while pf or ff:
                for src in (ff, pf):
                    if src:
                        fillers.append(src.pop(0))
            if SEQFILL:
                for _, un in fillers:
                    un()
                fillers = []
            avs, rcs = attend(st, q2_cur, q2b_cur, rst["k2"], rst["k2b"],
                              rst["vaug"], fillers)
            pending_fin = finish_units(st, avs, rcs,
                                       last=(g == len(blocks) - 1))
            q2_cur = q2_next
            q2b_cur = q2b_next if g + 1 < len(blocks) else None
        for _, un in pending_fin:
            un()

    nc.compile()
    return nc


# ---------------- host-side helpers ----------------

def core_rows(core):
    hg = core % 4
    heads = [4 * hg + h for h in range(HPC)]
    qk_rows = np.concatenate(
        [np.concatenate([64 * h + 2 * np.arange(32),
                         64 * h + 2 * np.arange(32) + 1])
         for h in heads])                                     # [256]
    v_rows = np.concatenate([64 * h + np.arange(64) for h in heads])  # [256]
    return heads, qk_rows, v_rows


def make_in_map(core, x, W_q, W_k, W_v, W_o, positions, theta, S,
                mm_dtype="bf16"):
    import ml_dtypes

    b = core // 4
    _, qk_rows, v_rows = core_rows(core)
    pos = np.asarray(positions).astype(np.float32)
    inv_freq = np.float32(theta) ** (
        -np.arange(0, 32, dtype=np.float32) * np.float32(2.0 / DK))
    ang = pos[None, :] * inv_freq[:, None]          # [32, S]
    cosb = np.tile(np.cos(ang), (4, 1)).astype(np.float32)
    sin32 = np.sin(ang).astype(np.float32)
    # sn_sw: multiplies ps BEFORE the 32-block swap; rows [ +s, -s ] per
    # 64-block so that swap(ps*sn_sw) = [-ps_o*s, +ps_e*s]
    snsw = np.tile(np.concatenate([sin32, -sin32], axis=0), (2, 1))
    tri = np.triu(np.ones((KB, KB), np.float32))
    tri4 = np.tile(tri, (1, HPC))

    def pmajor(wt):   # [d, ncol] -> [128, (d//128)*ncol] partition-major
        d, ncol = wt.shape
        return wt.reshape(d // 128, 128, ncol).transpose(1, 0, 2).reshape(
            128, (d // 128) * ncol)

    bf = ml_dtypes.bfloat16
    f8 = ml_dtypes.float8_e4m3 if FP8AV else ml_dtypes.bfloat16
    c = np.ascontiguousarray
    return {
        "xT": c(np.asarray(x[b]).T).astype(bf),
        "wqT": c(pmajor(np.asarray(W_q)[qk_rows].T)).astype(bf),
        "wkT": c(pmajor(np.asarray(W_k)[qk_rows].T)).astype(bf),
        "wvT": c(pmajor(np.asarray(W_v)[v_rows].T)).astype(bf),
        "woT": c(pmajor(np.asarray(W_o)[:, v_rows].T)).astype(bf),
        "cosb": c(cosb[:, :S]).astype(bf),
        "snswb": c(snsw[:, :S]).astype(bf),
        "tri4": c(tri4).astype(f8),
    }


# ---------------- public entry point ----------------

S_FULL = 2048
MM_DTYPE = "bf16"
_NC_CACHE = {}


def _get_nc():
    if "nc" not in _NC_CACHE:
        _NC_CACHE["nc"] = build_nc(S=S_FULL, mm_dtype=MM_DTYPE)
    return _NC_CACHE["nc"]


def kernel(x, W_q, W_k, W_v, W_o, token_positions, max_seq_len, theta):
    from concourse import bass_utils

    x = np.asarray(x, dtype=np.float32)
    W_q = np.asarray(W_q, dtype=np.float32)
    W_k = np.asarray(W_k, dtype=np.float32)
    W_v = np.asarray(W_v, dtype=np.float32)
    W_o = np.asarray(W_o, dtype=np.float32)
    positions = np.asarray(token_positions)
    theta_f = float(np.asarray(theta))

    nc = _get_nc()
    in_maps = [
        make_in_map(c, x, W_q, W_k, W_v, W_o, positions, theta_f, S_FULL,
                    mm_dtype=MM_DTYPE)
        for c in range(8)
    ]
    res = bass_utils.run_bass_kernel_spmd(nc, in_maps, core_ids=list(range(8)))
    outs = [np.asarray(res.results[c]["out"]).astype(np.float32)
            for c in range(8)]
    full = np.empty((2, S_FULL, 1024), np.float32)
    for b in range(2):
        full[b] = np.sum([outs[4 * b + i] for i in range(4)], axis=0,
                         dtype=np.float32)
    return full
